# revision 1
# baseline (speedup 1.0000x reference)
"""Transformer block (LN->MHA->LN->MLP, causal) on 8 Trainium2 NeuronCores.

Sharding: core = (batch b in {0,1}) x (position c in {0..3}).  Each core
computes the full output for 512 query tokens of its batch: 256-token
chunks {c, c+4} (of 8 chunks).  K/V are computed redundantly per core for
all 2048 tokens of its batch, which avoids any collective (an on-chip
AllReduce measures ~300us for this payload; the redundant K/V matmuls are
far cheaper).  LayerNorm affine params are folded into the projection
weights host-side.  Matmuls run in fp16 with fp32 PSUM accumulation.
Softmax skips the max-subtraction (scores are bounded ~|3.5|) and gets its
denominators via a ones-column appended to V; 1/denominator is broadcast
across partitions on the otherwise-idle GpSimd engine.

Layout scheme (all chosen so no operand ever needs a transpose beyond the
two LN outputs): activations that feed matmul contractions are kept
channel-major ("T" suffix, [C on partitions, tokens free]); attention
probabilities live as [keys, queries]; V and the MLP residual stream stay
token-major.
"""

import sys
import os

for p in ("/opt/trn_rl_repo", os.path.expanduser("~/.axon_site/_ro/trn_rl_repo")):
    if os.path.isdir(p) and p not in sys.path:
        sys.path.insert(0, p)

import numpy as np

import concourse.bass as bass
import concourse.tile as tile
import concourse.mybir as mybir
from concourse import bacc
from concourse.bass_utils import run_bass_kernel_spmd
from concourse.masks import make_identity

F32 = mybir.dt.float32
F16 = mybir.dt.float16
AF = mybir.ActivationFunctionType

B, T, C = 2, 2048, 1024
H, D, FF = 16, 64, 4 * 1024
P = 128
NT = T // P            # 16 token tiles per batch
NC_ = C // P           # 8 channel tiles
NFF = FF // P          # 32 ff tiles
NSLOT = 2              # query slots per core (256 tokens each)
SLOTW = 256            # slot width in tokens
QTOK = NSLOT * SLOTW   # 512 query tokens per core
NTOKT = QTOK // P      # 4 token tiles per core
EPS = 1e-5

_cache = {}


def _build_program(reps=1):
    """Build the SPMD program (identical on all 8 cores; data differs).

    reps>1 unrolls the whole computation N times in one NEFF — used only
    for wall-clock benchmarking (run-time difference between reps values
    isolates pure on-device execution time).
    """
    nc = bacc.Bacc("TRN2", target_bir_lowering=False, debug=False,
                   enable_asserts=False, num_devices=8)

    xb_d = nc.dram_tensor("xb", [T, C], F32, kind="ExternalInput").ap()
    xq_d = nc.dram_tensor("xq", [QTOK, C], F32, kind="ExternalInput").ap()
    mk_d = nc.dram_tensor("mk", [P, 4, 4, SLOTW], F16,
                          kind="ExternalInput").ap()
    # weights arrive pre-tiled partition-major so every slab DMA is one
    # contiguous segment per partition (HWDGE descriptor generation cost
    # scales with segment count)
    wq_d = nc.dram_tensor("wq", [P, 2, NC_, 512], F16,
                          kind="ExternalInput").ap()
    wk_d = nc.dram_tensor("wk", [P, 2, NC_, 512], F16,
                          kind="ExternalInput").ap()
    wv_d = nc.dram_tensor("wv", [P, 2, NC_, 512], F16,
                          kind="ExternalInput").ap()
    wo_d = nc.dram_tensor("wo", [P, 2, NC_, 512], F16,
                          kind="ExternalInput").ap()
    w1_d = nc.dram_tensor("w1", [P, 8, NC_, 512], F16,
                          kind="ExternalInput").ap()
    w2_d = nc.dram_tensor("w2", [P, 2, 2, NFF // 2, 512], F16,
                          kind="ExternalInput").ap()
    out_d = nc.dram_tensor("out", [QTOK, C], F32, kind="ExternalOutput").ap()

    with tile.TileContext(nc) as tc:
        for _ in range(reps):
            _emit(tc, nc, xb_d, xq_d, mk_d, wq_d, wk_d, wv_d, wo_d, w1_d,
                  w2_d, out_d)
    nc.compile()
    return nc


def _ln_tile(nc, pool, x_ap, out_ap, eps_tile):
    """LayerNorm one [128, C] fp32 tile -> fp16 out (no affine)."""
    sub = 512
    nsub = C // sub
    stats = pool.tile([P, nsub, 6], F32, tag="ln_stats")
    xr = x_ap.rearrange("p (n s) -> p n s", s=sub)
    for i in range(nsub):
        nc.vector.bn_stats(out=stats[:, i, :], in_=xr[:, i, :])
    mv = pool.tile([P, 2], F32, tag="ln_mv")
    nc.vector.bn_aggr(out=mv[:, :], in_=stats[:, :, :])
    rstd = pool.tile([P, 1], F32, tag="ln_rstd")
    nc.scalar.activation(out=rstd[:, :], in_=mv[:, 1:2], func=AF.Sqrt,
                         bias=eps_tile[:, :])
    nc.vector.reciprocal(out=rstd[:, :], in_=rstd[:, :])
    nc.vector.tensor_scalar(out=out_ap, in0=x_ap,
                            scalar1=mv[:, 0:1], scalar2=rstd[:, :],
                            op0=mybir.AluOpType.subtract,
                            op1=mybir.AluOpType.mult)


def _wslab(ap_4d, half):
    """Pre-tiled weight slab: select half -> [128, 8, 512] one-segment AP."""
    return ap_4d[:, half, :, :]


def _emit(tc, nc, xb_d, xq_d, mk_d, wq_d, wk_d, wv_d, wo_d, w1_d, w2_d, out_d):
    from contextlib import ExitStack
    ctx = ExitStack()
    with ctx:
        singles = ctx.enter_context(tc.tile_pool(name="singles", bufs=1))
        big = ctx.enter_context(tc.tile_pool(name="big", bufs=1))
        pkv = ctx.enter_context(tc.tile_pool(name="pkv", bufs=2))
        pv = ctx.enter_context(tc.tile_pool(name="pv", bufs=1))
        pq = ctx.enter_context(tc.tile_pool(name="pq", bufs=1))
        phq = ctx.enter_context(tc.tile_pool(name="phq", bufs=1))
        pxq = ctx.enter_context(tc.tile_pool(name="pxq", bufs=1))
        pattn = ctx.enter_context(tc.tile_pool(name="pattn", bufs=1))
        wf = ctx.enter_context(tc.tile_pool(name="wf", bufs=3))
        work = ctx.enter_context(tc.tile_pool(name="work", bufs=3))
        ev = ctx.enter_context(tc.tile_pool(name="ev", bufs=4))
        evy = ctx.enter_context(tc.tile_pool(name="evy", bufs=4))
        mmps = ctx.enter_context(tc.tile_pool(name="mmps", bufs=3,
                                              space="PSUM"))
        avps = ctx.enter_context(tc.tile_pool(name="avps", bufs=2,
                                              space="PSUM"))
        smps = ctx.enter_context(tc.tile_pool(name="smps", bufs=3,
                                              space="PSUM"))

        ident = singles.tile([P, P], F16)
        make_identity(nc, ident)
        eps_t = singles.tile([P, 1], F32)
        nc.vector.memset(eps_t, EPS)
        masks = singles.tile([P, 4, 4, SLOTW], F16)

        # Persistent big buffers (tag-aliased across phases)
        hT = big.tile([P, NC_, T], F16, tag="bigA")          # 32KB/part
        kTa = pkv.tile([P, NC_ // 2, T], F16, tag="kt")      # 16KB/part
        kTb = pkv.tile([P, NC_ // 2, T], F16, tag="kt")      # 16KB/part
        kThalves = (kTa, kTb)
        vA = pv.tile([P, NT, H * (D + 1)], F16, tag="va")    # 32.5KB/part
        qT = pq.tile([P, NC_, QTOK], F16, tag="qt")          # 8KB
        hqT = phq.tile([P, NC_, QTOK], F16, tag="hq")        # 8KB
        xqs = pxq.tile([P, NTOKT, C], F32, tag="xq")         # 16KB

        # ---- Phase 0: load x, LN1, transpose -> hT / hqT; ones into vA;
        # the V projection is fused per token tile so PE has dense work
        # while LayerNorm runs on DVE/ACT.
        # xq tiles + q projection first: qT matmuls fill the PE pipe while
        # the batch-wide x tiles stream in behind them.
        for st in range(NTOKT):
            xt = work.tile([P, C], F32, tag="x_in")
            nc.sync.dma_start(out=xt[:, :], in_=xq_d[st * P:(st + 1) * P, :])
            nc.scalar.copy(out=xqs[:, st, :], in_=xt[:, :])
            ht = work.tile([P, C], F16, tag="h_ln")
            _ln_tile(nc, work, xt[:, :], ht[:, :], eps_t)
            for ct in range(NC_):
                tp = smps.tile([P, P], F16, tag="sm")
                nc.tensor.transpose(tp[:, :], ht[:, ct * P:(ct + 1) * P],
                                    ident[:, :])
                nc.scalar.copy(out=hqT[:, ct, st * P:(st + 1) * P],
                               in_=tp[:, :])
        for hf in range(2):
            wqf = wf.tile([P, NC_, 512], F16, tag="wfull")
            nc.sync.dma_start(out=wqf[:, :, :],
                              in_=_wslab(wq_d, hf))
            for mj in range(4):
                mt = hf * 4 + mj
                ps = mmps.tile([P, 512], F32, tag="mm")
                for ct in range(NC_):
                    nc.tensor.matmul(ps[:, :],
                                     wqf[:, ct, mj * P:(mj + 1) * P],
                                     hqT[:, ct, :],
                                     start=(ct == 0), stop=(ct == NC_ - 1))
                nc.vector.tensor_copy(out=qT[:, mt, :], in_=ps[:, :])

        wvf = []
        for bk in range(2):
            wvf_half = wf.tile([P, NC_, 512], F16, tag="wfull")
            nc.sync.dma_start(out=wvf_half[:, :, :],
                              in_=_wslab(wv_d, bk))
            wvf.append(wvf_half)
        for tt in range(NT):
            xt = work.tile([P, C], F32, tag="x_in")
            dma_eng = nc.sync if tt < 3 else nc.gpsimd
            dma_eng.dma_start(out=xt[:, :], in_=xb_d[tt * P:(tt + 1) * P, :])
            if tt == 0:
                nc.vector.memset(
                    vA[:, :, :].rearrange("p t (h c) -> p t h c",
                                          c=D + 1)[:, :, :, D:], 1.0)
                nc.gpsimd.dma_start(out=masks[:, :, :, :],
                                    in_=mk_d[:, :, :, :])
            ht = work.tile([P, C], F16, tag="h_ln")
            _ln_tile(nc, work, xt[:, :], ht[:, :], eps_t)
            for ct in range(NC_):
                tp = smps.tile([P, P], F16, tag="sm")
                nc.tensor.transpose(tp[:, :], ht[:, ct * P:(ct + 1) * P],
                                    ident[:, :])
                nc.scalar.copy(out=hT[:, ct, tt * P:(tt + 1) * P],
                               in_=tp[:, :])
            for bk in range(2):
                ps = mmps.tile([P, 512], F32, tag="mm")
                for ct in range(NC_):
                    nc.tensor.matmul(ps[:, :],
                                     hT[:, ct, tt * P:(tt + 1) * P],
                                     wvf[bk][:, ct, :],
                                     start=(ct == 0), stop=(ct == NC_ - 1))
                dst = vA[:, tt, bk * 8 * (D + 1):(bk + 1) * 8 * (D + 1)]
                dst = dst.rearrange("p (h c) -> p h c", c=D + 1)[:, :, 0:D]
                nc.vector.tensor_copy(out=dst, in_=ps[:, :].rearrange(
                    "p (h c) -> p h c", c=D))

        # ---- Phase 3: kT Mtile production + attention for its head pair ----
        OT = phq.tile([P, NC_, QTOK], F16, tag="hq")     # aliases hqT
        for hf in range(2):
            wkf = wf.tile([P, NC_, 512], F16, tag="wfull")
            nc.sync.dma_start(out=wkf[:, :, :],
                              in_=_wslab(wk_d, hf))
            for mj in range(4):
                mt = hf * 4 + mj
                for ch in range(4):
                    ps = mmps.tile([P, 512], F32, tag="mm")
                    for ct in range(NC_):
                        nc.tensor.matmul(
                            ps[:, :],
                            wkf[:, ct, mj * P:(mj + 1) * P],
                            hT[:, ct, ch * 512:(ch + 1) * 512],
                            start=(ct == 0), stop=(ct == NC_ - 1))
                    nc.vector.tensor_copy(
                        out=kThalves[mt // 4][:, mt % 4,
                                              ch * 512:(ch + 1) * 512],
                        in_=ps[:, :])
                # attention for the two heads living in kT Mtile `mt`;
                # 4-kt score groups span two PSUM banks -> one exp per
                # [128, 1024]
                for h in (2 * mt, 2 * mt + 1):
                    pt = h // 2
                    r0 = (h % 2) * D
                    for s in range(NSLOT):
                        ngrp = 4 + 4 * s
                        av = avps.tile([D + 1, SLOTW], F32, tag="av")
                        for g in range(ngrp):
                            st = smps.tile([P, 2, SLOTW], F32, tag="sm")
                            for j in range(2):
                                kt = 2 * g + j
                                nc.tensor.matmul(
                                    st[:, j, :],
                                    kThalves[pt // 4][r0:r0 + D, pt % 4,
                                                      kt * P:(kt + 1) * P],
                                    qT[r0:r0 + D, pt,
                                       s * SLOTW:(s + 1) * SLOTW],
                                    start=(j == 0), stop=(j == 1))
                            e = ev.tile([P, 2, SLOTW], F16, tag="e")
                            nc.scalar.activation(out=e[:, :, :],
                                                 in_=st[:, :, :],
                                                 func=AF.Exp, scale=0.125)
                            bg, jh = g // 2, g % 2
                            if s == 0 or bg >= 2:
                                nc.vector.tensor_mul(
                                    e[:, :, :], e[:, :, :],
                                    masks[:, bg, 2 * jh:2 * jh + 2, :])
                            for j in range(2):
                                kt = 2 * g + j
                                nc.tensor.matmul(
                                    av[:, :],
                                    vA[:, kt, h * (D + 1):(h + 1) * (D + 1)],
                                    e[:, j, :],
                                    start=(kt == 0), stop=(kt == 2 * ngrp - 1))
                        rec = work.tile([1, SLOTW], F32, tag="rec")
                        nc.vector.reciprocal(out=rec[:, :], in_=av[D:D + 1, :])
                        bco = work.tile([D, SLOTW], F32, tag="bco")
                        nc.gpsimd.partition_broadcast(bco[:, :], rec[:, :])
                        nc.vector.tensor_mul(
                            OT[r0:r0 + D, pt, s * SLOTW:(s + 1) * SLOTW],
                            av[0:D, :], bco[:, :])

        # ---- Phase 5: out-proj + residual ----
        x2s = pv.tile([P, NTOKT, C], F32, tag="va")      # aliases vA
        for bk in range(2):
            wof = wf.tile([P, NC_, 512], F16, tag="wfull")
            nc.sync.dma_start(out=wof[:, :, :],
                              in_=_wslab(wo_d, bk))
            for s in range(NTOKT):
                ps = mmps.tile([P, 512], F32, tag="mm")
                for ct in range(NC_):
                    nc.tensor.matmul(ps[:, :],
                                     OT[:, ct, s * P:(s + 1) * P],
                                     wof[:, ct, :],
                                     start=(ct == 0), stop=(ct == NC_ - 1))
                nc.vector.tensor_add(x2s[:, s, bk * 512:(bk + 1) * 512],
                                     ps[:, :],
                                     xqs[:, s, bk * 512:(bk + 1) * 512])

        # ---- Phase 6: LN2 + transpose -> h2T ----
        h2T = pattn.tile([P, NC_, QTOK], F16, tag="at")
        for s in range(NTOKT):
            h2 = work.tile([P, C], F16, tag="h_ln")
            _ln_tile(nc, work, x2s[:, s, :], h2[:, :], eps_t)
            for ct in range(NC_):
                tp = smps.tile([P, P], F16, tag="sm")
                nc.tensor.transpose(tp[:, :], h2[:, ct * P:(ct + 1) * P],
                                    ident[:, :])
                nc.scalar.copy(out=h2T[:, ct, s * P:(s + 1) * P],
                               in_=tp[:, :])

        # ---- Phase 7: MLP up + GELU -> mT ----
        mT = big.tile([P, NFF, QTOK], F16, tag="bigA")   # aliases hT
        for mg in range(8):          # groups of 4 ff-tiles
            w1c = wf.tile([P, NC_, 512], F16, tag="wfull")
            nc.sync.dma_start(out=w1c[:, :, :],
                              in_=_wslab(w1_d, mg))
            for j in range(4):
                mt = mg * 4 + j
                ps = mmps.tile([P, 512], F32, tag="mm")
                for ct in range(NC_):
                    nc.tensor.matmul(ps[:, :],
                                     w1c[:, ct, j * P:(j + 1) * P],
                                     h2T[:, ct, :],
                                     start=(ct == 0), stop=(ct == NC_ - 1))
                nc.scalar.activation(out=mT[:, mt, :], in_=ps[:, :],
                                     func=AF.Gelu)

        # ---- Phase 8: MLP down + residual -> out ----
        # W2 streams as 4 quarters [2048, 512] double-buffered through the
        # two kT slots (freed mid-attention, so the first loads prefetch
        # early).
        NFH = NFF // 2
        for bk in range(2):
            w2q = []
            for fh in range(2):
                w2qt = pkv.tile([P, NFH, 512], F16, tag="kt")
                nc.sync.dma_start(out=w2qt[:, :, :],
                                  in_=w2_d[:, bk, fh, :, :])
                w2q.append(w2qt)
            for s in range(NTOKT):
                ps = mmps.tile([P, 512], F32, tag="mm")
                for ft in range(NFF):
                    nc.tensor.matmul(ps[:, :],
                                     mT[:, ft, s * P:(s + 1) * P],
                                     w2q[ft // NFH][:, ft % NFH, :],
                                     start=(ft == 0), stop=(ft == NFF - 1))
                yt = evy.tile([P, 512], F32, tag="y")
                nc.vector.tensor_add(yt[:, :], ps[:, :],
                                     x2s[:, s, bk * 512:(bk + 1) * 512])
                nc.sync.dma_start(
                    out=out_d[s * P:(s + 1) * P, bk * 512:(bk + 1) * 512],
                    in_=yt[:, :])


def _prep_inputs(x, Wq, Wk, Wv, Wo, bo, W1, b1, W2, b2, g1, be1, g2, be2):
    """Fold LN affines into weights; build per-core input maps."""
    f16 = np.float16

    def tile_ccol(w, nhalf):
        # [1024, nhalf*512] -> [p, half, ct, n]; element (ct*128+p, half*512+n)
        return np.ascontiguousarray(
            w.reshape(NC_, P, nhalf, 512).transpose(1, 2, 0, 3).astype(f16))

    Wq_ = tile_ccol(g1[:, None] * Wq, 2)
    Wk_ = tile_ccol(g1[:, None] * Wk, 2)
    Wv_ = tile_ccol(g1[:, None] * Wv, 2)
    Wo_ = tile_ccol(Wo, 2)
    W1_ = tile_ccol(g2[:, None] * W1, 8)
    # W2 [4096, 1024] -> [p, bk, fh, ft, n]; element (fh*2048+ft*128+p,
    # bk*512+n)
    W2_ = np.ascontiguousarray(
        W2.reshape(2, NFF // 2, P, 2, 512).transpose(2, 3, 0, 1, 4)
        .astype(f16))
    for name, v in (("be1@W", be1), ("bo", bo), ("b1", b1), ("b2", b2),
                    ("be2@W", be2)):
        if np.any(v):
            raise NotImplementedError(f"nonzero bias {name} not supported")

    in_maps = []
    for core in range(8):
        b, c = core // 4, core % 4
        xb = np.ascontiguousarray(x[b])
        # query chunks of 256 tokens: chunk c and chunk c+4 (of 8)
        chunks = [c + 4 * s for s in range(NSLOT)]
        xq = np.concatenate([xb[ch * SLOTW:(ch + 1) * SLOTW] for ch in chunks],
                            axis=0)
        # masks[p, bg, j, q]: big-group bg covers kts 4bg..4bg+3; key
        # token = 128*(4bg+j) + p; the group belongs to slot 0 for bg<2
        # else slot 1 (query token = 256*chunk(slot) + q)
        mk = np.zeros((P, 4, 4, SLOTW), np.float16)
        kk = np.arange(P)[:, None]
        qq = np.arange(SLOTW)[None, :]
        for bg in range(4):
            ch = chunks[0] if bg < 2 else chunks[1]
            for j in range(4):
                kt = 4 * bg + j
                mk[:, bg, j, :] = (kt * P + kk <= ch * SLOTW + qq)
        in_maps.append(dict(xb=xb, xq=np.ascontiguousarray(xq), mk=mk,
                            wq=Wq_, wk=Wk_, wv=Wv_, wo=Wo_, w1=W1_, w2=W2_))
    return in_maps


def kernel(x, Wq, Wk, Wv, Wo, bo, W1, b1, W2, b2, g1, be1, g2, be2,
           _trace=False):
    args = (x, Wq, Wk, Wv, Wo, bo, W1, b1, W2, b2, g1, be1, g2, be2)
    args = tuple(np.asarray(a, np.float32) for a in args)
    in_maps = _prep_inputs(*args)

    if "nc" not in _cache:
        _cache["nc"] = _build_program()
    nc = _cache["nc"]

    res = run_bass_kernel_spmd(nc, in_maps, core_ids=list(range(8)),
                               trace=_trace)
    _cache["last_results"] = res

    out = np.empty((B, T, C), np.float32)
    for core in range(8):
        b, c = core // 4, core % 4
        o = res.results[core]["out"]
        for s in range(NSLOT):
            ch = c + 4 * s
            out[b, ch * SLOTW:(ch + 1) * SLOTW, :] = \
                o[s * SLOTW:(s + 1) * SLOTW, :]
    return out


if __name__ == "__main__":
    rng = np.random.default_rng(0)
    x = rng.standard_normal((B, T, C), dtype=np.float32)
    sc = 0.02
    W = lambda *s: (rng.standard_normal(s, dtype=np.float32) * sc)
    out = kernel(x, W(C, C), W(C, C), W(C, C), W(C, C), np.zeros(C, np.float32),
                 W(C, FF), np.zeros(FF, np.float32), W(FF, C),
                 np.zeros(C, np.float32), np.ones(C, np.float32),
                 np.zeros(C, np.float32), np.ones(C, np.float32),
                 np.zeros(C, np.float32))
    print("out", out.shape, out.dtype, np.abs(out).max())



# revision 5
# speedup vs baseline: 3.5059x; 3.5059x over previous
"""Transformer block (LN->MHA->LN->MLP, causal) on 8 NeuronCores.

Instruction-minimal design for this environment (cost ~ per instruction):
head-parallel attention (core = batch x head-group of 4), exact causal
trimming, LayerNorms folded into the projections via linearity
(W^T ln(x) = r * (W^T x + c1 (x) (-mu)), c1 = column sums of W),
DMA-transposes instead of PE transposes, partial out-projection +
4-way ReduceScatter to go head-sharded -> token-sharded,
token-parallel MLP.  ~1400 instructions/core vs ~4800 in the
query-parallel baseline.
"""

import sys
import os

for p in ("/opt/trn_rl_repo", os.path.expanduser("~/.axon_site/_ro/trn_rl_repo")):
    if os.path.isdir(p) and p not in sys.path:
        sys.path.insert(0, p)

import numpy as np

import concourse.bass as bass
import concourse.tile as tile
import concourse.mybir as mybir
from concourse import bacc
from concourse.bass_utils import run_bass_kernel_spmd
from concourse.masks import make_identity

F32 = mybir.dt.float32
F16 = mybir.dt.float16
AF = mybir.ActivationFunctionType
OP = mybir.AluOpType

B, T, C = 2, 2048, 1024
H, D, FF = 16, 64, 4 * 1024
P = 128
NC_ = C // P           # 8 channel tiles
HG = 4                 # heads per core
TSH = 512              # token shard per core
EPS = 1e-5

_cache = {}
SIM_GELU = False   # interp has no Gelu; use tanh approx when simulating
DEBUG = False      # add intermediate DRAM outputs for sim debugging

# score chunk boundaries per q-slab s4 (max 6 PSUM banks per chunk,
# diagonal k-tiles 4*s4..4*s4+3 always within one chunk)
CHUNKS = {
    0: [(0, 4)],
    1: [(0, 4), (4, 4)],
    2: [(0, 6), (6, 6)],
    3: [(0, 6), (6, 6), (12, 4)],
}


def _build_program(reps=1):
    nc = bacc.Bacc("TRN2", target_bir_lowering=False, debug=False,
                   enable_asserts=False, num_devices=8)

    xt_d = nc.dram_tensor("xt", [P, NC_, T], F16, kind="ExternalInput").ap()
    xs_d = nc.dram_tensor("xs", [P, 4, C], F16, kind="ExternalInput").ap()
    wqkv_d = nc.dram_tensor("wqkv", [P, NC_, 3, 2 * P], F16,
                            kind="ExternalInput").ap()
    c1s_d = nc.dram_tensor("c1s", [1, 3, 2 * P], F16,
                           kind="ExternalInput").ap()
    wo_d = nc.dram_tensor("wo", [D, HG, C], F16, kind="ExternalInput").ap()
    # w1 slot 8 row 0 of each half holds the c1w1 column sums
    w1_d = nc.dram_tensor("w1", [P, 2, NC_ + 1, FF // 2], F16,
                          kind="ExternalInput").ap()
    w2_d = nc.dram_tensor("w2", [P, FF // P, C], F16,
                          kind="ExternalInput").ap()
    msk_d = nc.dram_tensor("msk", [P, 4, TSH], F16, kind="ExternalInput").ap()
    out_d = nc.dram_tensor("out", [TSH, C], F32, kind="ExternalOutput").ap()
    dbg = {}
    if DEBUG:
        for nm, shp, dt_ in (("dQT", [P, 2, T], F16), ("dKT", [P, 2, T], F16),
                             ("dV", [P, T // P, HG, D + 1], F16),
                             ("dOT", [D, HG, 4, TSH], F16),
                             ("dpart", [P, T // P, C], F16),
                             ("dxa", [P, 4, C], F16),
                             ("dx2T", [P, NC_, TSH], F16),
                             ("dnegmu", [1, T], F16),
                             ("drb", [1, T], F16),
                             ("dm", [P, 4, FF], F16),
                             ("dmT", [P, FF // P, TSH], F16)):
            dbg[nm] = nc.dram_tensor(nm, shp, dt_, kind="ExternalOutput").ap()

    with tile.TileContext(nc) as tc:
        for _ in range(reps):
            _emit(tc, nc, xt_d, xs_d, wqkv_d, c1s_d, wo_d, w1_d, w2_d,
                  msk_d, out_d, dbg)
    nc.compile()
    return nc


def _emit(tc, nc, xt_d, xs_d, wqkv_d, c1s_d, wo_d, w1_d, w2_d, msk_d, out_d,
          dbg={}):
    from contextlib import ExitStack
    ctx = ExitStack()
    with ctx:
        sing = ctx.enter_context(tc.tile_pool(name="sing", bufs=1))
        bigA = ctx.enter_context(tc.tile_pool(name="bigA", bufs=1))
        bigB = ctx.enter_context(tc.tile_pool(name="bigB", bufs=1))
        wox = ctx.enter_context(tc.tile_pool(name="wox", bufs=1))
        dram = ctx.enter_context(tc.tile_pool(name="dram", bufs=1,
                                              space="DRAM"))

        # ---- persistent SBUF (~70KB/partition) ----
        xt = bigA.tile([P, NC_, T], F16, tag="bigA")          # 32KB/part
        QT = sing.tile([P, 2, T], F16)                        # 8KB
        KT = sing.tile([P, 2, T], F16)                        # 8KB
        V = sing.tile([P, T // P, HG, D + 1], F16)            # 8.1KB
        xs = sing.tile([P, 4, C], F16)                        # 8KB
        msk = sing.tile([P, 4, TSH], F16)                     # 4KB
        xa = sing.tile([P, 4, C], F16)                        # 8KB
        x2T = sing.tile([P, NC_, TSH], F16)                   # 8KB
        wo = wox.tile([D, HG, C], F16, tag="wox")            # 8KB
        negmu = sing.tile([P, T], F16)                        # 4KB (row 0)
        r_b = sing.tile([P, T], F16)                          # 4KB
        negmu2 = sing.tile([P, TSH], F16)                     # 1KB (row 0)
        ones = sing.tile([P, 1], F16)
        ident = sing.tile([P, P], F16)
        eps1 = sing.tile([1, 1], F32)

        nc.vector.memset(eps1, EPS)
        nc.vector.memset(ones, 1.0)
        make_identity(nc, ident)

        nc.sync.dma_start(out=xt[:, :, :], in_=xt_d[:, :, :])
        nc.sync.dma_start(out=xs[:, :, :], in_=xs_d[:, :, :])
        nc.sync.dma_start(out=msk[:, :, :], in_=msk_d[:, :, :])
        nc.sync.dma_start(out=wo[:, :, :], in_=wo_d[:, :, :])

        # ---- LN1 stats (channels on partitions x 8 ct tiles) ----
        with tc.tile_pool(name="st_sb", bufs=1) as st_sb, \
             tc.tile_pool(name="st_ps", bufs=1, space="PSUM") as st_ps:
            acc = st_sb.tile([P, T], F16, tag="acc")
            tmp = st_sb.tile([P, T], F16, tag="tmp")
            nc.vector.tensor_add(acc[:, :], xt[:, 0, :], xt[:, 1, :])
            for c in range(2, NC_):
                nc.vector.tensor_add(acc[:, :], acc[:, :], xt[:, c, :])
            xsum = st_ps.tile([1, 4, TSH], F32)
            for i in range(4):
                nc.tensor.matmul(xsum[:, i, :], ones[:, :],
                                 acc[:, i * TSH:(i + 1) * TSH],
                                 start=True, stop=True)
            nc.scalar.square(acc[:, :], xt[:, 0, :])
            for c in range(1, NC_):
                nc.scalar.square(tmp[:, :], xt[:, c, :])
                nc.vector.tensor_add(acc[:, :], acc[:, :], tmp[:, :])
            x2sum = st_ps.tile([1, 4, TSH], F32)
            for i in range(4):
                nc.tensor.matmul(x2sum[:, i, :], ones[:, :],
                                 acc[:, i * TSH:(i + 1) * TSH],
                                 start=True, stop=True)
            mu = st_sb.tile([1, T], F32, tag="mu")
            var = st_sb.tile([1, T], F32, tag="var")
            rr = st_sb.tile([1, T], F32, tag="rr")
            xs_f = xsum[:, :, :].rearrange("p a b -> p (a b)")
            x2_f = x2sum[:, :, :].rearrange("p a b -> p (a b)")
            nc.vector.tensor_scalar(out=mu[:, :], in0=xs_f, scalar1=1.0 / C,
                                    scalar2=None, op0=OP.mult)
            nc.vector.tensor_mul(var[:, :], mu[:, :], mu[:, :])
            nc.vector.scalar_tensor_tensor(out=var[:, :], in0=x2_f,
                                           scalar=1.0 / C, in1=var[:, :],
                                           op0=OP.mult, op1=OP.subtract)
            nc.scalar.activation(out=var[:, :], in_=var[:, :], func=AF.Sqrt,
                                 bias=eps1[:, :])
            nc.vector.reciprocal(out=rr[:, :], in_=var[:, :])
            nc.vector.tensor_scalar(out=negmu[0:1, :], in0=mu[:, :],
                                    scalar1=-1.0, scalar2=None, op0=OP.mult)
            rr16 = st_sb.tile([1, T], F16, tag="rr16")
            nc.vector.tensor_copy(out=rr16[:, :], in_=rr[:, :])
            nc.gpsimd.partition_broadcast(r_b[:, :], rr16[:, :])

        # ---- QKV projections (LN folded) ----
        dsts = (QT, KT, None)
        with tc.tile_pool(name="qk_sb", bufs=1) as qk_sb:
          with tc.tile_pool(name="qkv_ps", bufs=2, space="PSUM") as qkv_ps:
            wqkv = qk_sb.tile([P, NC_, 3, 2 * P], F16, tag="wqkv")
            c1s = qk_sb.tile([P, 3, 2 * P], F16, tag="c1s")
            VT = qk_sb.tile([P, 2, T], F16, tag="VT")
            nc.sync.dma_start(out=wqkv[:, :, :, :], in_=wqkv_d[:, :, :, :])
            nc.sync.dma_start(out=c1s[0:1, :, :], in_=c1s_d[:, :, :])
            for j in range(3):
                for m in range(2):
                    ps = qkv_ps.tile([P, 4, TSH], F32)
                    for i in range(4):
                        for c in range(NC_):
                            nc.tensor.matmul(ps[:, i, :],
                                             wqkv[:, c, j, m * P:(m + 1) * P],
                                             xt[:, c, i * TSH:(i + 1) * TSH],
                                             start=(c == 0), stop=False)
                        nc.tensor.matmul(ps[:, i, :],
                                         c1s[0:1, j, m * P:(m + 1) * P],
                                         negmu[0:1, i * TSH:(i + 1) * TSH],
                                         start=False, stop=True)
                    dst = dsts[j] if j < 2 else VT
                    nc.vector.tensor_mul(
                        dst[:, m, :].rearrange("p (a b) -> p a b", a=4),
                        ps[:, :, :],
                        r_b[:, :].rearrange("p (a b) -> p a b", a=4))
          # V -> token-major via PE transposes (DMA-transpose from a
          # 64-partition source is broken on the real path)
          with tc.tile_pool(name="tp_ps", bufs=2, space="PSUM") as tp_ps:
            nc.vector.memset(V[:, :, :, D:D + 1], 1.0)
            for mh in range(2):
                for kt in range(T // P):
                    tp = tp_ps.tile([P, P], F16)
                    nc.tensor.transpose(tp[:, :],
                                        VT[:, mh, kt * P:(kt + 1) * P],
                                        ident[:, :])
                    for j in range(2):
                        nc.scalar.copy(
                            out=V[:, kt, 2 * mh + j, 0:D],
                            in_=tp[:, j * D:(j + 1) * D])

        if dbg:
            nc.sync.dma_start(out=dbg["dQT"][:, :, :], in_=QT[:, :, :])
            nc.sync.dma_start(out=dbg["dKT"][:, :, :], in_=KT[:, :, :])
            nc.sync.dma_start(out=dbg["dV"][:, :, :, :], in_=V[:, :, :, :])
            nc.sync.dma_start(out=dbg["dnegmu"][:, :], in_=negmu[0:1, :])
            nc.sync.dma_start(out=dbg["drb"][:, :], in_=r_b[0:1, :])

        # ---- attention (exact causal, per head / 512-query slab) ----
        OT = bigB.tile([D, HG, 4, TSH], F16, tag="bigB")
        with tc.tile_pool(name="ep", bufs=2) as ep, \
             tc.tile_pool(name="sc_ps", bufs=1, space="PSUM") as sc_ps, \
             tc.tile_pool(name="av_ps", bufs=2, space="PSUM") as av_ps:
            for h in range(HG):
                qh = QT[(h % 2) * D:(h % 2) * D + D, h // 2, :]
                kh = KT[(h % 2) * D:(h % 2) * D + D, h // 2, :]
                for s4 in range(4):
                    av = av_ps.tile([D + 1, TSH], F32)
                    nkt_tot = 4 * (s4 + 1)
                    for (kt0, nkt) in CHUNKS[s4]:
                        sc = sc_ps.tile([P, 6, TSH], F32)
                        for i in range(nkt):
                            kt = kt0 + i
                            nc.tensor.matmul(
                                sc[:, i, :],
                                kh[:, kt * P:(kt + 1) * P],
                                qh[:, s4 * TSH:(s4 + 1) * TSH],
                                start=True, stop=True)
                        e = ep.tile([P, 6, TSH], F16, tag="e")
                        nc.scalar.activation(out=e[:, 0:nkt, :],
                                             in_=sc[:, 0:nkt, :],
                                             func=AF.Exp, scale=0.125)
                        dg0 = 4 * s4
                        if kt0 <= dg0 and dg0 + 3 <= kt0 + nkt - 1:
                            o = dg0 - kt0
                            nc.vector.tensor_mul(e[:, o:o + 4, :],
                                                 e[:, o:o + 4, :],
                                                 msk[:, :, :])
                        for i in range(nkt):
                            kt = kt0 + i
                            nc.tensor.matmul(av[:, :], V[:, kt, h, :],
                                             e[:, i, :],
                                             start=(kt == 0),
                                             stop=(kt == nkt_tot - 1))
                    rec = ep.tile([1, TSH], F32, tag="rec")
                    nc.vector.reciprocal(out=rec[:, :], in_=av[D:D + 1, :])
                    bco = ep.tile([D, TSH], F32, tag="bco")
                    nc.gpsimd.partition_broadcast(bco[:, :], rec[:, :])
                    nc.vector.tensor_mul(OT[:, h, s4, :], av[0:D, :],
                                         bco[:, :])

        # ---- partial out-proj (my 4 heads, all 2048 tokens) ----
        part = bigA.tile([P, T // P, C], F16, tag="bigA")   # aliases xt
        with tc.tile_pool(name="op_ps", bufs=2, space="PSUM") as op_ps:
            for t in range(T // P):
                ps = op_ps.tile([P, 2, TSH], F32)
                for n in range(2):
                    for h in range(HG):
                        lhsT = OT[:, h, t // 4, (t % 4) * P:(t % 4) * P + P]
                        nc.tensor.matmul(ps[:, n, :], lhsT,
                                         wo[:, h, n * TSH:(n + 1) * TSH],
                                         start=(h == 0), stop=(h == HG - 1))
                nc.vector.tensor_copy(
                    out=part[:, t, :].rearrange("p (a b) -> p a b", a=2),
                    in_=ps[:, :, :])

        if dbg:
            nc.sync.dma_start(out=dbg["dOT"][:, :, :, :], in_=OT[:, :, :, :])
            nc.sync.dma_start(out=dbg["dpart"][:, :, :], in_=part[:, :, :])

        # ---- ReduceScatter partials -> my 512-token shard; residual ----
        pt_d = dram.tile([T, C], F16, tag="pt")
        psh_d = dram.tile([TSH, C], F16, tag="psh")
        nc.sync.dma_start(
            out=pt_d[:, :].rearrange("(t p) n -> p t n", p=P),
            in_=part[:, :, :])
        nc.gpsimd.collective_compute(
            "ReduceScatter", OP.add,
            replica_groups=[[0, 1, 2, 3], [4, 5, 6, 7]],
            ins=[pt_d[:, :]], outs=[psh_d[:, :]])
        po = bigB.tile([P, 4, C], F16, tag="bigB")          # aliases OT
        nc.sync.dma_start(
            out=po[:, :, :],
            in_=psh_d[:, :].rearrange("(t p) n -> p t n", p=P))
        nc.vector.tensor_add(xa[:, :, :], po[:, :, :], xs[:, :, :])

        # ---- x2T = xa transposed; LN2 stats in transposed space ----
        # (dense DMA-transpose dst + copy; gapped dst is not trusted on HW)
        with tc.tile_pool(name="xtp", bufs=2) as xtp:
            for t in range(4):
                dtmp = xtp.tile([P, NC_, P], F16, tag="d")
                nc.sync.dma_start(out=dtmp[:, :, :], in_=xa[:, t, :],
                                  transpose=True)
                nc.vector.tensor_copy(out=x2T[:, :, t * P:(t + 1) * P],
                                      in_=dtmp[:, :, :])
        with tc.tile_pool(name="l2_sb", bufs=1) as l2_sb, \
             tc.tile_pool(name="l2_ps", bufs=1, space="PSUM") as l2_ps:
            acc = l2_sb.tile([P, TSH], F16, tag="acc")
            tmp = l2_sb.tile([P, TSH], F16, tag="tmp")
            nc.vector.tensor_add(acc[:, :], x2T[:, 0, :], x2T[:, 1, :])
            for c in range(2, NC_):
                nc.vector.tensor_add(acc[:, :], acc[:, :], x2T[:, c, :])
            xsum2 = l2_ps.tile([1, TSH], F32)
            nc.tensor.matmul(xsum2[:, :], ones[:, :], acc[:, :],
                             start=True, stop=True)
            nc.scalar.square(acc[:, :], x2T[:, 0, :])
            for c in range(1, NC_):
                nc.scalar.square(tmp[:, :], x2T[:, c, :])
                nc.vector.tensor_add(acc[:, :], acc[:, :], tmp[:, :])
            x2sum2 = l2_ps.tile([1, TSH], F32)
            nc.tensor.matmul(x2sum2[:, :], ones[:, :], acc[:, :],
                             start=True, stop=True)
            mu2 = l2_sb.tile([1, TSH], F32, tag="mu2")
            va2 = l2_sb.tile([1, TSH], F32, tag="va2")
            rr2 = l2_sb.tile([1, TSH], F32, tag="rr2")
            nc.vector.tensor_scalar(out=mu2[:, :], in0=xsum2[:, :],
                                    scalar1=1.0 / C, scalar2=None,
                                    op0=OP.mult)
            nc.vector.tensor_mul(va2[:, :], mu2[:, :], mu2[:, :])
            nc.vector.scalar_tensor_tensor(out=va2[:, :], in0=x2sum2[:, :],
                                           scalar=1.0 / C, in1=va2[:, :],
                                           op0=OP.mult, op1=OP.subtract)
            nc.scalar.activation(out=va2[:, :], in_=va2[:, :], func=AF.Sqrt,
                                 bias=eps1[:, :])
            nc.vector.reciprocal(out=rr2[:, :], in_=va2[:, :])
            # negmu2 row0 = -mu2*r2 (rank-1 rhs); fold r2 into x2T
            nc.vector.tensor_mul(negmu2[0:1, :], mu2[:, :], rr2[:, :])
            nc.vector.tensor_scalar(out=negmu2[0:1, :], in0=negmu2[0:1, :],
                                    scalar1=-1.0, scalar2=None, op0=OP.mult)
            rr216 = l2_sb.tile([1, TSH], F16, tag="rr216")
            nc.vector.tensor_copy(out=rr216[:, :], in_=rr2[:, :])
            r2b = l2_sb.tile([P, TSH], F16, tag="r2b")
            nc.gpsimd.partition_broadcast(r2b[:, :], rr216[:, :])
            for c in range(NC_):
                nc.vector.tensor_mul(x2T[:, c, :], x2T[:, c, :], r2b[:, :])

        if dbg:
            nc.sync.dma_start(out=dbg["dxa"][:, :, :], in_=xa[:, :, :])
            nc.sync.dma_start(out=dbg["dx2T"][:, :, :], in_=x2T[:, :, :])

        # ---- MLP up + GELU (LN2 folded), m -> mT, down + residual ----
        m = bigA.tile([P, 4, FF], F16, tag="bigA")          # aliases part
        mT = bigB.tile([P, FF // P, TSH], F16, tag="bigB")  # aliases po
        with tc.tile_pool(name="wp", bufs=2) as wp:
            with tc.tile_pool(name="up_ps", bufs=2, space="PSUM") as up_ps:
                for half in range(2):
                    w1h = wp.tile([P, NC_ + 1, FF // 2], F16, tag="w")
                    nc.sync.dma_start(out=w1h[:, :, :],
                                      in_=w1_d[:, half, :, :])
                    for t in range(4):
                        ps = up_ps.tile([P, 4, TSH], F32)
                        for f in range(4):
                            for c in range(NC_):
                                nc.tensor.matmul(
                                    ps[:, f, :], x2T[:, c, t * P:(t + 1) * P],
                                    w1h[:, c, f * TSH:(f + 1) * TSH],
                                    start=(c == 0), stop=False)
                            nc.tensor.matmul(
                                ps[:, f, :],
                                negmu2[0:1, t * P:(t + 1) * P],
                                w1h[0:1, NC_, f * TSH:(f + 1) * TSH],
                                start=False, stop=True)
                        mdst = m[:, t,
                                 half * (FF // 2):(half + 1) * (FF // 2)] \
                            .rearrange("p (a b) -> p a b", a=4)
                        if not SIM_GELU:
                            nc.scalar.activation(out=mdst, in_=ps[:, :, :],
                                                 func=AF.Gelu)
                        else:
                            gp = wox.tile([P, 4, TSH], F16, tag="wox")
                            nc.scalar.square(gp[:, :, :], ps[:, :, :])
                            nc.vector.tensor_mul(gp[:, :, :], gp[:, :, :],
                                                 ps[:, :, :])
                            nc.vector.scalar_tensor_tensor(
                                out=gp[:, :, :], in0=gp[:, :, :],
                                scalar=0.044715, in1=ps[:, :, :],
                                op0=OP.mult, op1=OP.add)
                            nc.scalar.activation(out=gp[:, :, :],
                                                 in_=gp[:, :, :],
                                                 func=AF.Tanh,
                                                 scale=0.7978845608)
                            nc.vector.tensor_scalar(
                                out=mdst, in0=ps[:, :, :],
                                scalar1=0.5, scalar2=None, op0=OP.mult)
                            nc.vector.scalar_tensor_tensor(
                                out=mdst, in0=gp[:, :, :], scalar=1.0,
                                in1=mdst, op0=OP.add, op1=OP.mult)
            for t in range(4):
                dtmp = wox.tile([P, FF // P, P], F16, tag="wox")
                nc.sync.dma_start(out=dtmp[:, :, :], in_=m[:, t, :],
                                  transpose=True)
                nc.vector.tensor_copy(out=mT[:, :, t * P:(t + 1) * P],
                                      in_=dtmp[:, :, :])
            if dbg:
                nc.sync.dma_start(out=dbg["dm"][:, :, :], in_=m[:, :, :])
                nc.sync.dma_start(out=dbg["dmT"][:, :, :], in_=mT[:, :, :])
            NFH = FF // P // 2
            w2a = wp.tile([P, NFH, C], F16, tag="w")
            w2b = wp.tile([P, NFH, C], F16, tag="w")
            nc.sync.dma_start(out=w2a[:, :, :], in_=w2_d[:, 0:NFH, :])
            nc.sync.dma_start(out=w2b[:, :, :], in_=w2_d[:, NFH:2 * NFH, :])
            NFC = FF // P
            with tc.tile_pool(name="dn_ps", bufs=2, space="PSUM") as dn_ps:
                for t in range(4):
                    ps = dn_ps.tile([P, 2, TSH], F32)
                    for n in range(2):
                        for fc in range(NFC):
                            w2h = w2a if fc < NFH else w2b
                            nc.tensor.matmul(
                                ps[:, n, :], mT[:, fc, t * P:(t + 1) * P],
                                w2h[:, fc % NFH, n * TSH:(n + 1) * TSH],
                                start=(fc == 0), stop=(fc == NFC - 1))
                    yt = wox.tile([P, 2, TSH], F32, tag="wox")
                    nc.vector.tensor_add(
                        yt[:, :, :], ps[:, :, :],
                        xa[:, t, :].rearrange("p (a b) -> p a b", a=2))
                    nc.sync.dma_start(
                        out=out_d[t * P:(t + 1) * P, :]
                        .rearrange("p (a b) -> p a b", a=2),
                        in_=yt[:, :, :])


def _prep_inputs(x, Wq, Wk, Wv, Wo, bo, W1, b1, W2, b2, g1, be1, g2, be2):
    f16 = np.float16
    for name, v in (("be1", be1), ("bo", bo), ("b1", b1), ("b2", b2),
                    ("be2", be2)):
        if np.any(v):
            raise NotImplementedError(f"nonzero bias {name} not supported")

    Wq_ = (g1[:, None] * Wq).astype(np.float32)
    Wk_ = (g1[:, None] * Wk).astype(np.float32)
    Wv_ = (g1[:, None] * Wv).astype(np.float32)
    W1_ = (g2[:, None] * W1).astype(np.float32)
    Wo_ = Wo.astype(np.float32)

    # w1 packed [128, 2, 9, 2048]; slot 8 row 0 = column sums
    w1t = W1_.reshape(NC_, P, 2, FF // 2).transpose(1, 2, 0, 3)
    w1p = np.zeros((P, 2, NC_ + 1, FF // 2), np.float32)
    w1p[:, :, 0:NC_, :] = w1t
    w1p[0, :, NC_, :] = W1_.sum(axis=0).reshape(2, FF // 2)
    w1p = w1p.astype(f16)

    w2_t = np.ascontiguousarray(
        W2.reshape(FF // P, P, C).transpose(1, 0, 2)).astype(f16)

    kk = np.arange(P)[:, None]
    qq = np.arange(TSH)[None, :]
    msk = np.zeros((P, 4, TSH), f16)
    for j in range(4):
        msk[:, j, :] = (j * P + kk <= qq)

    in_maps = []
    for core in range(8):
        b, g = core // 4, core % 4
        cs = slice(256 * g, 256 * (g + 1))
        xt = np.ascontiguousarray(
            x[b].T.reshape(NC_, P, T).transpose(1, 0, 2)).astype(f16)
        xsh = np.ascontiguousarray(
            x[b, TSH * g:TSH * (g + 1)].reshape(4, P, C)
            .transpose(1, 0, 2)).astype(f16)
        wqkv = np.stack([Wq_[:, cs], Wk_[:, cs], Wv_[:, cs]], axis=1)
        wqkv = np.ascontiguousarray(
            wqkv.reshape(NC_, P, 3, 256).transpose(1, 0, 2, 3)).astype(f16)
        c1s = np.stack([Wq_[:, cs].sum(0), Wk_[:, cs].sum(0),
                        Wv_[:, cs].sum(0)], axis=0)[None].astype(f16)
        wo_t = np.ascontiguousarray(
            Wo_[cs].reshape(HG, D, C).transpose(1, 0, 2)).astype(f16)
        in_maps.append(dict(xt=xt, xs=xsh, wqkv=wqkv, c1s=c1s, wo=wo_t,
                            w1=w1p, w2=w2_t, msk=msk))
    return in_maps


def kernel(x, Wq, Wk, Wv, Wo, bo, W1, b1, W2, b2, g1, be1, g2, be2,
           _trace=False):
    args = (x, Wq, Wk, Wv, Wo, bo, W1, b1, W2, b2, g1, be1, g2, be2)
    args = tuple(np.asarray(a, np.float32) for a in args)
    in_maps = _prep_inputs(*args)

    if "nc" not in _cache:
        _cache["nc"] = _build_program()
    nc = _cache["nc"]

    res = run_bass_kernel_spmd(nc, in_maps, core_ids=list(range(8)),
                               trace=_trace)
    _cache["last_results"] = res

    out = np.empty((B, T, C), np.float32)
    for core in range(8):
        b, g = core // 4, core % 4
        out[b, TSH * g:TSH * (g + 1), :] = res.results[core]["out"]
    return out


if __name__ == "__main__":
    rng = np.random.default_rng(0)
    x = rng.standard_normal((B, T, C), dtype=np.float32)
    sc = 0.02
    W = lambda *s: (rng.standard_normal(s, dtype=np.float32) * sc)
    out = kernel(x, W(C, C), W(C, C), W(C, C), W(C, C),
                 np.zeros(C, np.float32), W(C, FF), np.zeros(FF, np.float32),
                 W(FF, C), np.zeros(C, np.float32), np.ones(C, np.float32),
                 np.zeros(C, np.float32), np.ones(C, np.float32),
                 np.zeros(C, np.float32))
    print("out", out.shape, out.dtype, np.abs(out).max())
